# revision 9
# baseline (speedup 1.0000x reference)
"""DefocusLKPN Trainium2 kernel.

Computes, per batch element (reference semantics):
    r      = clip(alpha * defocus + tanh(unet[:,100]), 0, 3)
    disk_k = sigmoid(5*(r - dist_k))            (25 taps, 6 distinct dists)
    w_ck   = exp(l_ck) * disk_k                 (l = unet[:, :100] logits)
    out_c  = sum_k w_ck * patch_ck / sum_k w_ck + x_c

Identity used: sigmoid(z) = 0.5*(1 + tanh(z/2)); the global 0.5 cancels in
the num/den ratio, so w_ck = exp(l_ck) * (1 + tanh(2.5*(r - dist_k))).
The softmax normalizer of the reference also cancels exactly (the EPS clamp
in the reference is never active for |logits| of this distribution since the
center tap's disk mask is >= 0.5).

Sharding: pure data parallel, batch 16 -> 2 per core across 8 cores.

Per-core layout: partition dim = H (128); free dim packs (b, w) = 256 for
pixel planes and (k, b, w) for the 25-tap weight planes.  The 5x5 unfold is
realized as 5 row-shifted, column-padded copies of x in SBUF (vertical halo)
plus free-dim offsets (horizontal halo); the k-reduction runs on the tensor
engine as identity-matmul accumulation into PSUM.
"""

import sys

sys.path.insert(0, "/opt/trn_rl_repo")

import numpy as np

import concourse.bass as bass
import concourse.mybir as mybir
from concourse.tile import TileContext
from concourse.bass_utils import run_bass_kernel_spmd

F32 = mybir.dt.float32
AF = mybir.ActivationFunctionType
ALU = mybir.AluOpType

N_CORES = 8
B, C, H, W = 16, 4, 128, 128
BL = B // N_CORES            # 2 batch elements per core
KK = 25
BW = BL * W                  # 256: (b, w) free block
WP = W + 4                   # 132: padded width per (b, c) block

# distinct tap distances and the k -> dist-index map (k = (dy+2)*5 + (dx+2))
_D2_TO_IDX = {0: 0, 1: 1, 2: 2, 4: 3, 5: 4, 8: 5}
DISTS = [0.0, 1.0, np.sqrt(2.0), 2.0, np.sqrt(5.0), np.sqrt(8.0)]
# groups of taps sharing a dist, expressed as (base_k, [(step, count), ...])
# so the k offsets {base + i*s1 + j*s2} enumerate the group's taps.
GROUPS = [
    (0, 12, []),                    # dist 0:      {12}
    (1, 7, [(6, 2), (4, 2)]),       # dist 1:      {7, 11, 13, 17}
    (2, 6, [(10, 2), (2, 2)]),      # dist sqrt2:  {6, 8, 16, 18}
    (3, 2, [(12, 2), (8, 2)]),      # dist 2:      {2, 10, 14, 22}
    (4, 5, [(10, 2), (4, 2)]),      # dist sqrt5:  {5, 9, 15, 19}
    (4, 1, [(20, 2), (2, 2)]),      # dist sqrt5:  {1, 3, 21, 23}
    (5, 0, [(20, 2), (4, 2)]),      # dist sqrt8:  {0, 4, 20, 24}
]


def _split_wide_waits(nc, max_waits=1):
    """The walrus build here accepts at most one semaphore wait per
    instruction; move extra waits onto preceding Drains on the same engine."""
    n = 0
    for func in nc.m.functions:
        for bb in func.blocks:
            out = []
            changed = False
            for ins in bb.instructions:
                si = ins.sync_info
                if si is not None and si.on_wait and len(si.on_wait) > max_waits:
                    waits = list(si.on_wait)
                    keep, rest = waits[:max_waits], waits[max_waits:]
                    for i in range(0, len(rest), max_waits):
                        n += 1
                        out.append(
                            mybir.InstDrain(
                                name=f"splitwait-{n}",
                                opcode="Drain",
                                engine=ins.engine,
                                sync_info=mybir.SyncInfo(
                                    on_wait=list(rest[i : i + max_waits]),
                                    on_update=[],
                                ),
                            )
                        )
                    si.on_wait = keep
                    changed = True
                out.append(ins)
            if changed:
                bb.instructions = out
    return n


def _ap(t, extra_off, dims):
    """AP over tile/tensor `t` with the tile's partition dim, given free dims
    [[step, count], ...] in elements and an extra element offset."""
    return bass.AP(t.tensor, t.offset + extra_off, [list(t.ap[0])] + [list(d) for d in dims])


def _build():
    nc = bass.Bass("TRN2", num_devices=N_CORES)

    xl = nc.dram_tensor("x", [BL, C, H, W], F32, kind="ExternalInput")
    dfl = nc.dram_tensor("defocus", [BL, 1, H, W], F32, kind="ExternalInput")
    ul = nc.dram_tensor("unet", [BL, 4 * KK + 1, H, W], F32, kind="ExternalInput")
    al = nc.dram_tensor("alpha", [128, 1], F32, kind="ExternalInput")
    yl = nc.dram_tensor("y", [BL, C, H, W], F32, kind="ExternalOutput")

    ident_dram = nc.inline_tensor(np.eye(128, dtype=np.float32), name="ident")

    UCH = ul.shape[1]          # 101
    HWr = H * W                # plane stride in DRAM
    with TileContext(nc) as tc:
        with (
            tc.tile_pool(name="fix", bufs=1) as fix,
            tc.tile_pool(name="lp", bufs=2) as lp,
            tc.tile_pool(name="wp", bufs=2) as wpool,
            tc.tile_pool(name="mp", bufs=3) as mp,
            tc.tile_pool(name="op", bufs=2) as op,
            tc.tile_pool(name="ps", bufs=1, space="PSUM") as ps,
        ):
            # ---- constants / prologue ------------------------------------
            idt = fix.tile([128, 128], F32)
            nc.sync.dma_start(out=idt[:], in_=ident_dram[:])
            asb = fix.tile([128, 1], F32)
            nc.sync.dma_start(out=asb[:], in_=al[:])

            df = fix.tile([128, BW], F32)
            nc.sync.dma_start(
                out=df[:],
                in_=bass.AP(dfl, 0, [[W, H], [HWr, BL], [1, W]]),
            )
            u100 = fix.tile([128, BW], F32)
            nc.sync.dma_start(
                out=u100[:],
                in_=bass.AP(ul, 100 * HWr, [[W, H], [UCH * HWr, BL], [1, W]]),
            )
            xf = fix.tile([128, BL * C * W], F32)
            for b in range(BL):
                nc.sync.dma_start(
                    out=xf[:, b * C * W : (b + 1) * C * W],
                    in_=bass.AP(xl, b * C * HWr, [[W, H], [HWr, C], [1, W]]),
                )

            # radius = clip(alpha*defocus + tanh(u100), 0, 3)
            dtan = fix.tile([128, BW], F32)
            nc.scalar.activation(dtan[:], u100[:], AF.Tanh)
            r0 = fix.tile([128, BW], F32)
            nc.vector.scalar_tensor_tensor(r0[:], df[:], asb[:, :1], dtan[:], ALU.mult, ALU.add)
            rr = fix.tile([128, BW], F32)
            nc.vector.tensor_scalar(rr[:], r0[:], 0.0, 3.0, ALU.max, ALU.min)

            # t6[d] = tanh(2.5*r - 2.5*dist_d)   (6 planes, shared by all c)
            bt = fix.tile([128, 6], F32)
            for d in range(6):
                nc.gpsimd.memset(bt[:, d : d + 1], float(-2.5 * DISTS[d]))
            t6 = fix.tile([128, 6 * BW], F32)
            for d in range(6):
                nc.scalar.activation(
                    t6[:, d * BW : (d + 1) * BW], rr[:], AF.Tanh,
                    bias=bt[:, d : d + 1], scale=2.5,
                )

            # 5 row-shifted, column-padded copies of x: xs[dy][h, (b,c,132)]
            xs = []
            for dyi, dy in enumerate((-2, -1, 0, 1, 2)):
                xst = fix.tile([128, BL * C * WP], F32, name=f"xs{dyi}")
                nc.gpsimd.memset(xst[:], 0.0)
                lo, hi = max(0, -dy), 128 - max(0, dy)
                for b in range(BL):
                    src = xf.rearrange("p (b c w) -> p b c w", b=BL, c=C, w=W)[
                        lo + dy : hi + dy, b
                    ]
                    dst = xst.rearrange("p (b c wp) -> p b c wp", b=BL, c=C, wp=WP)[
                        lo:hi, b, :, 2 : 2 + W
                    ]
                    nc.sync.dma_start(out=dst, in_=src)
                xs.append(xst)

            # ---- per-channel main loop -----------------------------------
            nums, dens = [], []
            for c in range(C):
                num = ps.tile([128, BW], F32, name=f"num{c}")
                den = ps.tile([128, BW], F32, name=f"den{c}")
                nums.append(num)
                dens.append(den)

                l = lp.tile([128, KK * BW], F32, name="l")
                for b in range(BL):
                    nc.sync.dma_start(
                        out=_ap(l, b * W, [[BW, KK], [1, W]]),
                        in_=bass.AP(
                            ul, (c * KK + b * UCH) * HWr,
                            [[W, H], [HWr, KK], [1, W]],
                        ),
                    )
                # lexp = exp(l), in place
                nc.scalar.activation(l[:], l[:], AF.Exp)

                # w = (1 + t6[d(k)]) * lexp   (grouped by distinct dist;
                # engine APs allow at most 2 free dims, so split the 2x2
                # tap groups into pairs)
                w = wpool.tile([128, KK * BW], F32, name="w")
                for d_idx, base, dims in GROUPS:
                    if not dims:
                        bases, pair = [base], []
                    else:
                        (s1, n1), (s2, n2) = dims
                        bases = [base + i * s1 for i in range(n1)]
                        pair = [[s2 * BW, n2]]
                    for bk in bases:
                        gdims = pair + [[1, BW]]
                        bdims = [[0, n] for (_, n) in pair] + [[1, BW]]
                        nc.vector.scalar_tensor_tensor(
                            _ap(w, bk * BW, gdims),
                            _ap(t6, d_idx * BW, bdims),
                            1.0,
                            _ap(l, bk * BW, gdims),
                            ALU.add,
                            ALU.mult,
                        )

                # den_c += w_k ; num_c += w_k * xs_dy[:, c, dx:dx+W]
                for dy in range(5):
                    m = mp.tile([128, 5 * BW], F32, name="m")
                    for b in range(BL):
                        nc.vector.tensor_tensor(
                            _ap(m, b * W, [[BW, 5], [1, W]]),
                            _ap(w, dy * 5 * BW + b * W, [[BW, 5], [1, W]]),
                            _ap(xs[dy], c * WP + b * C * WP, [[1, 5], [1, W]]),
                            ALU.mult,
                        )
                    for dx in range(5):
                        k = dy * 5 + dx
                        nc.tensor.matmul(
                            num[:], idt[:], m[:, dx * BW : (dx + 1) * BW],
                            start=(k == 0), stop=(k == KK - 1),
                        )
                        nc.tensor.matmul(
                            den[:], idt[:], w[:, k * BW : (k + 1) * BW],
                            start=(k == 0), stop=(k == KK - 1),
                        )

            # ---- epilogue: out_c = num/den + x ---------------------------
            for c in range(C):
                rden = op.tile([128, BW], F32, name="rden")
                nc.vector.reciprocal(rden[:], dens[c][:])
                o1 = op.tile([128, BW], F32, name="o1")
                nc.vector.scalar_tensor_tensor(
                    o1[:], nums[c][:], 1.0, rden[:], ALU.bypass, ALU.mult
                )
                o2 = op.tile([128, BW], F32, name="o2")
                nc.vector.tensor_tensor(
                    o2[:], o1[:], _ap(xf, c * W, [[C * W, BL], [1, W]]), ALU.add
                )
                nc.sync.dma_start(
                    out=bass.AP(yl, c * HWr, [[W, H], [C * HWr, BL], [1, W]]),
                    in_=o2[:],
                )

    _split_wide_waits(nc)
    return nc


_NC_CACHE = None


def _get_nc():
    global _NC_CACHE
    if _NC_CACHE is None:
        _NC_CACHE = _build()
    return _NC_CACHE


def _make_in_maps(x, defocus_map, unet_out, alpha):
    x = np.ascontiguousarray(x, dtype=np.float32)
    defocus_map = np.ascontiguousarray(defocus_map, dtype=np.float32)
    unet_out = np.ascontiguousarray(unet_out, dtype=np.float32)
    alpha_b = np.full((128, 1), np.float32(np.asarray(alpha).reshape(-1)[0]))
    in_maps = []
    for core in range(N_CORES):
        s = slice(core * BL, (core + 1) * BL)
        in_maps.append(
            {
                "x": x[s],
                "defocus": defocus_map[s],
                "unet": unet_out[s],
                "alpha": alpha_b,
            }
        )
    return in_maps


def run(x, defocus_map, unet_out, alpha, **spmd_kwargs):
    """Run the kernel; returns (output, BassKernelResults)."""
    nc = _get_nc()
    in_maps = _make_in_maps(x, defocus_map, unet_out, alpha)
    res = run_bass_kernel_spmd(nc, in_maps, list(range(N_CORES)), **spmd_kwargs)
    out = np.concatenate([res.results[i]["y"] for i in range(N_CORES)], axis=0)
    return out.astype(np.float32), res


def kernel(x, defocus_map, unet_out, alpha):
    return run(x, defocus_map, unet_out, alpha)[0]


# revision 12
# speedup vs baseline: 1.3985x; 1.3985x over previous
"""DefocusLKPN Trainium2 kernel.

Computes, per batch element (reference semantics):
    r      = clip(alpha * defocus + tanh(unet[:,100]), 0, 3)
    disk_k = sigmoid(5*(r - dist_k))            (25 taps, 6 distinct dists)
    w_ck   = exp(l_ck) * disk_k                 (l = unet[:, :100] logits)
    out_c  = sum_k w_ck * patch_ck / sum_k w_ck + x_c

Identity used: sigmoid(z) = 0.5*(1 + tanh(z/2)); the global 0.5 cancels in
the num/den ratio, so w_ck = exp(l_ck) * (1 + tanh(2.5*(r - dist_k))).
The softmax normalizer of the reference also cancels exactly (the EPS clamp
in the reference is never active for logits of this scale since the center
tap's disk mask is >= 0.5).

Sharding: pure data parallel, batch 16 -> 2 per core across 8 cores.

Per-core layout: partition dim = H (128); free dim packs (b, w) = 256 for
pixel planes and (k, b, w) for the 25-tap weight planes.  The 5x5 unfold is
realized as 5 row-shifted, column-padded copies of x in SBUF (vertical halo)
plus free-dim offsets (horizontal halo); the k-reduction runs on the tensor
engine as identity-matmul accumulation into PSUM (bf16 operands, f32
accumulation).  Only the tap weights and patches are bf16; the radius chain,
the normalization (num/den) and the final '+ x' stay f32.
"""

import sys

sys.path.insert(0, "/opt/trn_rl_repo")

import numpy as np

import concourse.bass as bass
import concourse.mybir as mybir
from concourse.tile import TileContext
from concourse.bass_utils import run_bass_kernel_spmd

F32 = mybir.dt.float32
BF16 = mybir.dt.bfloat16
AF = mybir.ActivationFunctionType
ALU = mybir.AluOpType

# dtype of the tap-weight / patch pipeline (lexp, t6, w, xs, m, identity).
MM_DT = BF16

N_CORES = 8
B, C, H, W = 16, 4, 128, 128
BL = B // N_CORES            # 2 batch elements per core
KK = 25
BW = BL * W                  # 256: (b, w) free block
WP = W + 4                   # 132: padded width per (b, c) block

# distinct tap distances; k = (dy+2)*5 + (dx+2)
DISTS = [0.0, 1.0, np.sqrt(2.0), 2.0, np.sqrt(5.0), np.sqrt(8.0)]
# (dist_index, base_k, [(step, count), ...]): {base + i*s1 + j*s2} lists the
# taps sharing that dist.
GROUPS = [
    (0, 12, []),                    # dist 0:      {12}
    (1, 7, [(6, 2), (4, 2)]),       # dist 1:      {7, 11, 13, 17}
    (2, 6, [(10, 2), (2, 2)]),      # dist sqrt2:  {6, 8, 16, 18}
    (3, 2, [(12, 2), (8, 2)]),      # dist 2:      {2, 10, 14, 22}
    (4, 5, [(10, 2), (4, 2)]),      # dist sqrt5:  {5, 9, 15, 19}
    (4, 1, [(20, 2), (2, 2)]),      # dist sqrt5:  {1, 3, 21, 23}
    (5, 0, [(20, 2), (4, 2)]),      # dist sqrt8:  {0, 4, 20, 24}
]


def _split_wide_waits(nc, max_waits=1):
    """The walrus build here accepts at most one semaphore wait per
    instruction; move extra waits onto preceding Drains on the same engine."""
    n = 0
    for func in nc.m.functions:
        for bb in func.blocks:
            out = []
            changed = False
            for ins in bb.instructions:
                si = ins.sync_info
                if si is not None and si.on_wait and len(si.on_wait) > max_waits:
                    waits = list(si.on_wait)
                    keep, rest = waits[:max_waits], waits[max_waits:]
                    for i in range(0, len(rest), max_waits):
                        n += 1
                        out.append(
                            mybir.InstDrain(
                                name=f"splitwait-{n}",
                                opcode="Drain",
                                engine=ins.engine,
                                sync_info=mybir.SyncInfo(
                                    on_wait=list(rest[i : i + max_waits]),
                                    on_update=[],
                                ),
                            )
                        )
                    si.on_wait = keep
                    changed = True
                out.append(ins)
            if changed:
                bb.instructions = out
    return n


def _ap(t, extra_off, dims):
    """AP over tile `t` keeping its partition dim, with free dims
    [[step, count], ...] in elements and an extra element offset."""
    return bass.AP(t.tensor, t.offset + extra_off, [list(t.ap[0])] + [list(d) for d in dims])


def _build():
    nc = bass.Bass("TRN2", num_devices=N_CORES)

    xl = nc.dram_tensor("x", [BL, C, H, W], F32, kind="ExternalInput")
    dfl = nc.dram_tensor("defocus", [BL, 1, H, W], F32, kind="ExternalInput")
    ul = nc.dram_tensor("unet", [BL, 4 * KK + 1, H, W], F32, kind="ExternalInput")
    al = nc.dram_tensor("alpha", [128, 1], F32, kind="ExternalInput")
    yl = nc.dram_tensor("y", [BL, C, H, W], F32, kind="ExternalOutput")

    ident_np = np.eye(128)
    ident_dram = nc.inline_tensor(
        ident_np.astype(mybir.dt.np(MM_DT)), name="ident"
    )

    UCH = ul.shape[1]          # 101
    HWr = H * W                # plane stride in DRAM
    # round-robin issuing engines for the big logit loads: each engine's
    # HWDGE has its own queue, so this parallelizes the HBM streams.
    dma_engines = [nc.sync, nc.scalar, nc.gpsimd]

    with TileContext(nc) as tc:
        with (
            tc.tile_pool(name="fix", bufs=1) as fix,
            tc.tile_pool(name="lp", bufs=2) as lp,
            tc.tile_pool(name="ep", bufs=2) as ep,
            tc.tile_pool(name="wp", bufs=2) as wpool,
            tc.tile_pool(name="mp", bufs=3) as mp,
            tc.tile_pool(name="op", bufs=2) as op,
            tc.tile_pool(name="ps", bufs=1, space="PSUM") as ps,
        ):
            # ---- constants / prologue ------------------------------------
            idt = fix.tile([128, 128], MM_DT)
            nc.sync.dma_start(out=idt[:], in_=ident_dram[:])
            asb = fix.tile([128, 1], F32)
            nc.sync.dma_start(out=asb[:], in_=al[:])

            df = fix.tile([128, BW], F32)
            nc.sync.dma_start(
                out=df[:],
                in_=bass.AP(dfl, 0, [[W, H], [HWr, BL], [1, W]]),
            )
            u100 = fix.tile([128, BW], F32)
            nc.sync.dma_start(
                out=u100[:],
                in_=bass.AP(ul, 100 * HWr, [[W, H], [UCH * HWr, BL], [1, W]]),
            )
            xf = fix.tile([128, BL * C * W], F32)
            for b in range(BL):
                nc.sync.dma_start(
                    out=xf[:, b * C * W : (b + 1) * C * W],
                    in_=bass.AP(xl, b * C * HWr, [[W, H], [HWr, C], [1, W]]),
                )

            # radius = clip(alpha*defocus + tanh(u100), 0, 3)
            dtan = fix.tile([128, BW], F32)
            nc.scalar.activation(dtan[:], u100[:], AF.Tanh)
            r0 = fix.tile([128, BW], F32)
            nc.vector.scalar_tensor_tensor(r0[:], df[:], asb[:, :1], dtan[:], ALU.mult, ALU.add)
            rr = fix.tile([128, BW], F32)
            nc.vector.tensor_scalar(rr[:], r0[:], 0.0, 3.0, ALU.max, ALU.min)

            # t6[d] = tanh(2.5*r - 2.5*dist_d)   (6 planes, shared by all c)
            bt = fix.tile([128, 6], F32)
            for d in range(6):
                nc.gpsimd.memset(bt[:, d : d + 1], float(-2.5 * DISTS[d]))
            t6 = fix.tile([128, 6 * BW], MM_DT)
            for d in range(6):
                nc.scalar.activation(
                    t6[:, d * BW : (d + 1) * BW], rr[:], AF.Tanh,
                    bias=bt[:, d : d + 1], scale=2.5,
                )

            # x cast to the matmul dtype, then 5 row-shifted padded copies
            if MM_DT is F32:
                xb = xf
            else:
                xb = fix.tile([128, BL * C * W], MM_DT)
                nc.vector.tensor_copy(xb[:], xf[:])
            xs = []
            for dyi, dy in enumerate((-2, -1, 0, 1, 2)):
                xst = fix.tile([128, BL * C * WP], MM_DT, name=f"xs{dyi}")
                nc.gpsimd.memset(xst[:], 0.0)
                lo, hi = max(0, -dy), 128 - max(0, dy)
                for b in range(BL):
                    src = xb.rearrange("p (b c w) -> p b c w", b=BL, c=C, w=W)[
                        lo + dy : hi + dy, b
                    ]
                    dst = xst.rearrange("p (b c wp) -> p b c wp", b=BL, c=C, wp=WP)[
                        lo:hi, b, :, 2 : 2 + W
                    ]
                    nc.sync.dma_start(out=dst, in_=src)
                xs.append(xst)

            # ---- per-channel main loop -----------------------------------
            nums, dens = [], []
            for c in range(C):
                num = ps.tile([128, BW], F32, name=f"num{c}")
                den = ps.tile([128, BW], F32, name=f"den{c}")
                nums.append(num)
                dens.append(den)

                l = lp.tile([128, KK * BW], F32, name="l")
                for b in range(BL):
                    dma_eng = dma_engines[(c * BL + b) % len(dma_engines)]
                    dma_eng.dma_start(
                        out=_ap(l, b * W, [[BW, KK], [1, W]]),
                        in_=bass.AP(
                            ul, (c * KK + b * UCH) * HWr,
                            [[W, H], [HWr, KK], [1, W]],
                        ),
                    )
                lexp = ep.tile([128, KK * BW], MM_DT, name="lexp")
                nc.scalar.activation(lexp[:], l[:], AF.Exp)

                # w = (1 + t6[d(k)]) * lexp   (grouped by distinct dist;
                # engine APs allow at most 2 free dims -> split 2x2 groups)
                w = wpool.tile([128, KK * BW], MM_DT, name="w")
                for d_idx, base, dims in GROUPS:
                    if not dims:
                        bases, pair = [base], []
                    else:
                        (s1, n1), (s2, n2) = dims
                        bases = [base + i * s1 for i in range(n1)]
                        pair = [[s2 * BW, n2]]
                    for bk in bases:
                        gdims = pair + [[1, BW]]
                        bdims = [[0, n] for (_, n) in pair] + [[1, BW]]
                        nc.vector.scalar_tensor_tensor(
                            _ap(w, bk * BW, gdims),
                            _ap(t6, d_idx * BW, bdims),
                            1.0,
                            _ap(lexp, bk * BW, gdims),
                            ALU.add,
                            ALU.mult,
                        )

                # den_c += w_k ; num_c += w_k * xs_dy[:, c, dx:dx+W]
                for dy in range(5):
                    m = mp.tile([128, 5 * BW], MM_DT, name="m")
                    for b in range(BL):
                        nc.vector.tensor_tensor(
                            _ap(m, b * W, [[BW, 5], [1, W]]),
                            _ap(w, dy * 5 * BW + b * W, [[BW, 5], [1, W]]),
                            _ap(xs[dy], c * WP + b * C * WP, [[1, 5], [1, W]]),
                            ALU.mult,
                        )
                    for dx in range(5):
                        k = dy * 5 + dx
                        nc.tensor.matmul(
                            num[:], idt[:], m[:, dx * BW : (dx + 1) * BW],
                            start=(k == 0), stop=(k == KK - 1),
                        )
                        nc.tensor.matmul(
                            den[:], idt[:], w[:, k * BW : (k + 1) * BW],
                            start=(k == 0), stop=(k == KK - 1),
                        )

            # ---- epilogue: out_c = num/den + x ---------------------------
            for c in range(C):
                rden = op.tile([128, BW], F32, name="rden")
                nc.vector.reciprocal(rden[:], dens[c][:])
                o1 = op.tile([128, BW], F32, name="o1")
                nc.vector.scalar_tensor_tensor(
                    o1[:], nums[c][:], 1.0, rden[:], ALU.bypass, ALU.mult
                )
                o2 = op.tile([128, BW], F32, name="o2")
                nc.vector.tensor_tensor(
                    o2[:], o1[:], _ap(xf, c * W, [[C * W, BL], [1, W]]), ALU.add
                )
                nc.sync.dma_start(
                    out=bass.AP(yl, c * HWr, [[W, H], [C * HWr, BL], [1, W]]),
                    in_=o2[:],
                )

    _split_wide_waits(nc)
    return nc


_NC_CACHE = None


def _get_nc():
    global _NC_CACHE
    if _NC_CACHE is None:
        _NC_CACHE = _build()
    return _NC_CACHE


def _make_in_maps(x, defocus_map, unet_out, alpha):
    x = np.ascontiguousarray(x, dtype=np.float32)
    defocus_map = np.ascontiguousarray(defocus_map, dtype=np.float32)
    unet_out = np.ascontiguousarray(unet_out, dtype=np.float32)
    alpha_b = np.full((128, 1), np.float32(np.asarray(alpha).reshape(-1)[0]))
    in_maps = []
    for core in range(N_CORES):
        s = slice(core * BL, (core + 1) * BL)
        in_maps.append(
            {
                "x": x[s],
                "defocus": defocus_map[s],
                "unet": unet_out[s],
                "alpha": alpha_b,
            }
        )
    return in_maps


def run(x, defocus_map, unet_out, alpha, **spmd_kwargs):
    """Run the kernel; returns (output, BassKernelResults)."""
    nc = _get_nc()
    in_maps = _make_in_maps(x, defocus_map, unet_out, alpha)
    res = run_bass_kernel_spmd(nc, in_maps, list(range(N_CORES)), **spmd_kwargs)
    out = np.concatenate([res.results[i]["y"] for i in range(N_CORES)], axis=0)
    return out.astype(np.float32), res


def kernel(x, defocus_map, unet_out, alpha):
    return run(x, defocus_map, unet_out, alpha)[0]


# revision 13
# speedup vs baseline: 1.5294x; 1.0936x over previous
"""DefocusLKPN Trainium2 kernel.

Computes, per batch element (reference semantics):
    r      = clip(alpha * defocus + tanh(unet[:,100]), 0, 3)
    disk_k = sigmoid(5*(r - dist_k))            (25 taps, 6 distinct dists)
    w_ck   = exp(l_ck) * disk_k                 (l = unet[:, :100] logits)
    out_c  = sum_k w_ck * patch_ck / sum_k w_ck + x_c

Identity used: the global factor 2 of 2*sigmoid cancels in the num/den
ratio, so w_ck = exp(l_ck) * sigmoid(5*(r - dist_k)) works directly; sigmoid
(rather than 1 + tanh) keeps full relative precision for small disk weights
in fp16.
The softmax normalizer of the reference also cancels exactly (the EPS clamp
in the reference is never active for logits of this scale since the center
tap's disk mask is >= 0.5).

Sharding: pure data parallel, batch 16 -> 2 per core across 8 cores.

Per-core layout: partition dim = H (128); free dim packs (b, w) = 256 for
pixel planes and (k, b, w) for the 25-tap weight planes.  The 5x5 unfold is
realized as 5 row-shifted, column-padded copies of x in SBUF (vertical halo)
plus free-dim offsets (horizontal halo); the k-reduction runs on the tensor
engine as identity-matmul accumulation into PSUM (bf16 operands, f32
accumulation).  Only the tap weights and patches are bf16; the radius chain,
the normalization (num/den) and the final '+ x' stay f32.  fp16 (not bf16):
the 10-bit mantissa keeps the weighted-average error ~3e-4 of scale.
"""

import sys

sys.path.insert(0, "/opt/trn_rl_repo")

import numpy as np

import concourse.bass as bass
import concourse.mybir as mybir
from concourse.tile import TileContext
from concourse.bass_utils import run_bass_kernel_spmd

F32 = mybir.dt.float32
BF16 = mybir.dt.bfloat16
FP16 = mybir.dt.float16
AF = mybir.ActivationFunctionType
ALU = mybir.AluOpType

# dtype of the tap-weight / patch pipeline (lexp, s6, w, xs, m, identity).
MM_DT = FP16

N_CORES = 8
B, C, H, W = 16, 4, 128, 128
BL = B // N_CORES            # 2 batch elements per core
KK = 25
BW = BL * W                  # 256: (b, w) free block
WP = W + 4                   # 132: padded width per (b, c) block

# distinct tap distances; k = (dy+2)*5 + (dx+2)
DISTS = [0.0, 1.0, np.sqrt(2.0), 2.0, np.sqrt(5.0), np.sqrt(8.0)]
# (dist_index, base_k, [(step, count), ...]): {base + i*s1 + j*s2} lists the
# taps sharing that dist.
GROUPS = [
    (0, 12, []),                    # dist 0:      {12}
    (1, 7, [(6, 2), (4, 2)]),       # dist 1:      {7, 11, 13, 17}
    (2, 6, [(10, 2), (2, 2)]),      # dist sqrt2:  {6, 8, 16, 18}
    (3, 2, [(12, 2), (8, 2)]),      # dist 2:      {2, 10, 14, 22}
    (4, 5, [(10, 2), (4, 2)]),      # dist sqrt5:  {5, 9, 15, 19}
    (4, 1, [(20, 2), (2, 2)]),      # dist sqrt5:  {1, 3, 21, 23}
    (5, 0, [(20, 2), (4, 2)]),      # dist sqrt8:  {0, 4, 20, 24}
]


def _split_wide_waits(nc, max_waits=1):
    """The walrus build here accepts at most one semaphore wait per
    instruction; move extra waits onto preceding Drains on the same engine."""
    n = 0
    for func in nc.m.functions:
        for bb in func.blocks:
            out = []
            changed = False
            for ins in bb.instructions:
                si = ins.sync_info
                if si is not None and si.on_wait and len(si.on_wait) > max_waits:
                    waits = list(si.on_wait)
                    keep, rest = waits[:max_waits], waits[max_waits:]
                    for i in range(0, len(rest), max_waits):
                        n += 1
                        out.append(
                            mybir.InstDrain(
                                name=f"splitwait-{n}",
                                opcode="Drain",
                                engine=ins.engine,
                                sync_info=mybir.SyncInfo(
                                    on_wait=list(rest[i : i + max_waits]),
                                    on_update=[],
                                ),
                            )
                        )
                    si.on_wait = keep
                    changed = True
                out.append(ins)
            if changed:
                bb.instructions = out
    return n


def _ap(t, extra_off, dims):
    """AP over tile `t` keeping its partition dim, with free dims
    [[step, count], ...] in elements and an extra element offset."""
    return bass.AP(t.tensor, t.offset + extra_off, [list(t.ap[0])] + [list(d) for d in dims])


def _build():
    nc = bass.Bass("TRN2", num_devices=N_CORES)

    xl = nc.dram_tensor("x", [BL, C, H, W], F32, kind="ExternalInput")
    dfl = nc.dram_tensor("defocus", [BL, 1, H, W], F32, kind="ExternalInput")
    ul = nc.dram_tensor("unet", [BL, 4 * KK + 1, H, W], F32, kind="ExternalInput")
    al = nc.dram_tensor("alpha", [128, 1], F32, kind="ExternalInput")
    yl = nc.dram_tensor("y", [BL, C, H, W], F32, kind="ExternalOutput")

    ident_np = np.eye(128)
    ident_dram = nc.inline_tensor(
        ident_np.astype(mybir.dt.np(MM_DT)), name="ident"
    )

    UCH = ul.shape[1]          # 101
    HWr = H * W                # plane stride in DRAM
    # round-robin issuing engines for the big logit loads: each engine's
    # HWDGE has its own queue, so this parallelizes the HBM streams.
    dma_engines = [nc.sync, nc.scalar, nc.gpsimd]

    with TileContext(nc) as tc:
        with (
            tc.tile_pool(name="fix", bufs=1) as fix,
            tc.tile_pool(name="lp", bufs=2) as lp,
            tc.tile_pool(name="ep", bufs=2) as ep,
            tc.tile_pool(name="wp", bufs=2) as wpool,
            tc.tile_pool(name="mp", bufs=3) as mp,
            tc.tile_pool(name="op", bufs=2) as op,
            tc.tile_pool(name="ps", bufs=1, space="PSUM") as ps,
        ):
            # ---- constants / prologue ------------------------------------
            idt = fix.tile([128, 128], MM_DT)
            nc.sync.dma_start(out=idt[:], in_=ident_dram[:])
            asb = fix.tile([128, 1], F32)
            nc.sync.dma_start(out=asb[:], in_=al[:])

            df = fix.tile([128, BW], F32)
            nc.sync.dma_start(
                out=df[:],
                in_=bass.AP(dfl, 0, [[W, H], [HWr, BL], [1, W]]),
            )
            u100 = fix.tile([128, BW], F32)
            nc.sync.dma_start(
                out=u100[:],
                in_=bass.AP(ul, 100 * HWr, [[W, H], [UCH * HWr, BL], [1, W]]),
            )
            xf = fix.tile([128, BL * C * W], F32)
            for b in range(BL):
                nc.sync.dma_start(
                    out=xf[:, b * C * W : (b + 1) * C * W],
                    in_=bass.AP(xl, b * C * HWr, [[W, H], [HWr, C], [1, W]]),
                )

            # radius = clip(alpha*defocus + tanh(u100), 0, 3)
            dtan = fix.tile([128, BW], F32)
            nc.scalar.activation(dtan[:], u100[:], AF.Tanh)
            r0 = fix.tile([128, BW], F32)
            nc.vector.scalar_tensor_tensor(r0[:], df[:], asb[:, :1], dtan[:], ALU.mult, ALU.add)
            rr = fix.tile([128, BW], F32)
            nc.vector.tensor_scalar(rr[:], r0[:], 0.0, 3.0, ALU.max, ALU.min)

            # s6[d] = sigmoid(5*r - 5*dist_d)   (6 planes, shared by all c)
            bt = fix.tile([128, 6], F32)
            for d in range(6):
                nc.gpsimd.memset(bt[:, d : d + 1], float(-5.0 * DISTS[d]))
            s6 = fix.tile([128, 6 * BW], MM_DT)
            for d in range(6):
                nc.scalar.activation(
                    s6[:, d * BW : (d + 1) * BW], rr[:], AF.Sigmoid,
                    bias=bt[:, d : d + 1], scale=5.0,
                )

            # x cast to the matmul dtype, then 5 row-shifted padded copies
            if MM_DT is F32:
                xb = xf
            else:
                xb = fix.tile([128, BL * C * W], MM_DT)
                nc.vector.tensor_copy(xb[:], xf[:])
            xs = []
            for dyi, dy in enumerate((-2, -1, 0, 1, 2)):
                xst = fix.tile([128, BL * C * WP], MM_DT, name=f"xs{dyi}")
                nc.gpsimd.memset(xst[:], 0.0)
                lo, hi = max(0, -dy), 128 - max(0, dy)
                for b in range(BL):
                    src = xb.rearrange("p (b c w) -> p b c w", b=BL, c=C, w=W)[
                        lo + dy : hi + dy, b
                    ]
                    dst = xst.rearrange("p (b c wp) -> p b c wp", b=BL, c=C, wp=WP)[
                        lo:hi, b, :, 2 : 2 + W
                    ]
                    nc.sync.dma_start(out=dst, in_=src)
                xs.append(xst)

            # ---- per-channel main loop -----------------------------------
            nums, dens = [], []
            for c in range(C):
                num = ps.tile([128, BW], F32, name=f"num{c}")
                den = ps.tile([128, BW], F32, name=f"den{c}")
                nums.append(num)
                dens.append(den)

                l = lp.tile([128, KK * BW], F32, name="l")
                for b in range(BL):
                    dma_eng = dma_engines[(c * BL + b) % len(dma_engines)]
                    dma_eng.dma_start(
                        out=_ap(l, b * W, [[BW, KK], [1, W]]),
                        in_=bass.AP(
                            ul, (c * KK + b * UCH) * HWr,
                            [[W, H], [HWr, KK], [1, W]],
                        ),
                    )
                lexp = ep.tile([128, KK * BW], MM_DT, name="lexp")
                nc.scalar.activation(lexp[:], l[:], AF.Exp)

                # w = s6[d(k)] * lexp   (grouped by distinct dist;
                # engine APs allow at most 2 free dims -> split 2x2 groups)
                w = wpool.tile([128, KK * BW], MM_DT, name="w")
                for d_idx, base, dims in GROUPS:
                    if not dims:
                        bases, pair = [base], []
                    else:
                        (s1, n1), (s2, n2) = dims
                        bases = [base + i * s1 for i in range(n1)]
                        pair = [[s2 * BW, n2]]
                    for bk in bases:
                        gdims = pair + [[1, BW]]
                        bdims = [[0, n] for (_, n) in pair] + [[1, BW]]
                        nc.vector.tensor_tensor(
                            _ap(w, bk * BW, gdims),
                            _ap(s6, d_idx * BW, bdims),
                            _ap(lexp, bk * BW, gdims),
                            ALU.mult,
                        )

                # den_c += w_k ; num_c += w_k * xs_dy[:, c, dx:dx+W]
                for dy in range(5):
                    m = mp.tile([128, 5 * BW], MM_DT, name="m")
                    for b in range(BL):
                        nc.vector.tensor_tensor(
                            _ap(m, b * W, [[BW, 5], [1, W]]),
                            _ap(w, dy * 5 * BW + b * W, [[BW, 5], [1, W]]),
                            _ap(xs[dy], c * WP + b * C * WP, [[1, 5], [1, W]]),
                            ALU.mult,
                        )
                    for dx in range(5):
                        k = dy * 5 + dx
                        nc.tensor.matmul(
                            num[:], idt[:], m[:, dx * BW : (dx + 1) * BW],
                            start=(k == 0), stop=(k == KK - 1),
                        )
                        nc.tensor.matmul(
                            den[:], idt[:], w[:, k * BW : (k + 1) * BW],
                            start=(k == 0), stop=(k == KK - 1),
                        )

            # ---- epilogue: out_c = num/den + x ---------------------------
            for c in range(C):
                rden = op.tile([128, BW], F32, name="rden")
                nc.vector.reciprocal(rden[:], dens[c][:])
                o1 = op.tile([128, BW], F32, name="o1")
                nc.vector.scalar_tensor_tensor(
                    o1[:], nums[c][:], 1.0, rden[:], ALU.bypass, ALU.mult
                )
                o2 = op.tile([128, BW], F32, name="o2")
                nc.vector.tensor_tensor(
                    o2[:], o1[:], _ap(xf, c * W, [[C * W, BL], [1, W]]), ALU.add
                )
                nc.sync.dma_start(
                    out=bass.AP(yl, c * HWr, [[W, H], [C * HWr, BL], [1, W]]),
                    in_=o2[:],
                )

    _split_wide_waits(nc)
    return nc


_NC_CACHE = None


def _get_nc():
    global _NC_CACHE
    if _NC_CACHE is None:
        _NC_CACHE = _build()
    return _NC_CACHE


def _make_in_maps(x, defocus_map, unet_out, alpha):
    x = np.ascontiguousarray(x, dtype=np.float32)
    defocus_map = np.ascontiguousarray(defocus_map, dtype=np.float32)
    unet_out = np.ascontiguousarray(unet_out, dtype=np.float32)
    alpha_b = np.full((128, 1), np.float32(np.asarray(alpha).reshape(-1)[0]))
    in_maps = []
    for core in range(N_CORES):
        s = slice(core * BL, (core + 1) * BL)
        in_maps.append(
            {
                "x": x[s],
                "defocus": defocus_map[s],
                "unet": unet_out[s],
                "alpha": alpha_b,
            }
        )
    return in_maps


def run(x, defocus_map, unet_out, alpha, **spmd_kwargs):
    """Run the kernel; returns (output, BassKernelResults)."""
    nc = _get_nc()
    in_maps = _make_in_maps(x, defocus_map, unet_out, alpha)
    res = run_bass_kernel_spmd(nc, in_maps, list(range(N_CORES)), **spmd_kwargs)
    out = np.concatenate([res.results[i]["y"] for i in range(N_CORES)], axis=0)
    return out.astype(np.float32), res


def kernel(x, defocus_map, unet_out, alpha):
    return run(x, defocus_map, unet_out, alpha)[0]
